# revision 9
# baseline (speedup 1.0000x reference)
"""CRF Viterbi decode on 8 Trainium2 NeuronCores.

Strategy (data parallel over batch):
  - 64 batches sharded 8-per-core; the (T+2)^2 transition matrix replicated.
  - Each core runs the sequential Viterbi forward recurrence for its 8
    sequences entirely on the vector engine, with the exact same float
    rounding order as the reference (cur = fl(fl(feat + trans) + part)),
    producing a bitwise-identical partition history part_hist[t, b, :].
  - Per-core layout: partitions = (b4:4, tag:32); per step one fused
    scalar_tensor_tensor add builds cur[(b4,i),(b2,j)] and two
    tensor_reduce(max, apply_transpose=True) ops do the 32x32 block
    transpose + max-over-i in one instruction each, writing part_t back
    in per-partition (b4,j) layout so the recurrence needs no extra
    data movement.
  - feats are pre-combined with transitions in bulk: replicated-broadcast
    DMAs build FREP[(b4,i), (b2,tau,j)] chunks and one wide STT per chunk
    computes FT = fl(feat + trans).
  - The backpointer reconstruction (O(B*S*T) gathers/argmaxes vs the
    device's O(B*S*T^2) DP) runs on host in numpy with the identical
    rounding, reproducing the reference decode exactly, including the
    mask/length handling.
"""

import numpy as np

B, S, T = 64, 512, 32
NCORES = 8
BPC = B // NCORES          # batches per core
P = 128
START, END = T - 2, T - 1
CH = 32                    # time-chunk for FT build
NCHUNK = S // CH

_PROGRAM_CACHE = {}


def _build_program():
    import concourse.mybir as mybir
    from concourse import bacc, tile

    AL = mybir.AluOpType
    F32 = mybir.dt.float32
    X = mybir.AxisListType.X

    nc = bacc.Bacc("TRN2", target_bir_lowering=False, debug=False)
    feats_d = nc.dram_tensor("feats", [BPC, S, T], F32, kind="ExternalInput").ap()
    trans_d = nc.dram_tensor("trans", [T, T], F32, kind="ExternalInput").ap()
    out_d = nc.dram_tensor("parthist", [P, S * 2], F32, kind="ExternalOutput").ap()

    with tile.TileContext(nc) as tc:
        with (
            tc.tile_pool(name="const", bufs=1) as cpool,
            tc.tile_pool(name="ft", bufs=1) as ftpool,
            tc.tile_pool(name="frep", bufs=3) as freppool,
            tc.tile_pool(name="work", bufs=2) as wpool,
        ):
            transi = cpool.tile([P, 2 * T], F32, tag="transi")
            transstart = cpool.tile([P, 1], F32, tag="transstart")
            f0t = cpool.tile([P, 2], F32, tag="f0t")
            parthist = cpool.tile([P, S * 2], F32, tag="parthist")

            # TRANSI[(b4,i),(b2,j)]=trans[i,j]; TRANSSTART[(b4,j)]=trans[START,j];
            # F0T[(b4,j),b2]=feats[b4*2+b2,0,j]  -- per-b4 2D-dest DMAs
            for b4 in range(4):
                sl = slice(b4 * 32, (b4 + 1) * 32)
                nc.sync.dma_start(
                    transi[sl, :].rearrange("p (b2 j) -> p b2 j", j=T),
                    trans_d.unsqueeze(1).broadcast_to([T, 2, T]))
                nc.sync.dma_start(
                    transstart[sl, :], trans_d[START, :].unsqueeze(1))
                nc.sync.dma_start(
                    f0t[sl, :],
                    feats_d[b4 * 2:b4 * 2 + 2, 0, :].rearrange("b j -> j b"))

            # part0 = fl(f0 + trans[START])
            nc.vector.scalar_tensor_tensor(
                out=parthist[:, 0:2], in0=f0t[:], scalar=0.0,
                in1=transstart[:].broadcast_to([P, 2]),
                op0=AL.bypass, op1=AL.add)

            # FT chunks: fl(feat + trans), i-orientation
            ft_tiles = []
            tr_v = transi[:].rearrange("p (b2 j) -> p b2 j", j=T)
            for c in range(NCHUNK):
                frep = freppool.tile([P, 2 * CH * T], F32, tag="frep")
                for b4 in range(4):
                    for b2 in range(2):
                        b = b4 * 2 + b2
                        src = (feats_d[b, c * CH:(c + 1) * CH, :]
                               .rearrange("t j -> (t j)").unsqueeze(0)
                               .broadcast_to([32, CH * T]))
                        nc.sync.dma_start(
                            frep[b4 * 32:(b4 + 1) * 32,
                                 b2 * CH * T:(b2 + 1) * CH * T], src)
                ft = ftpool.tile([P, 2 * CH * T], F32, tag=f"ft{c}")
                for b2 in range(2):
                    sl = slice(b2 * CH * T, (b2 + 1) * CH * T)
                    nc.vector.scalar_tensor_tensor(
                        out=ft[:, sl].rearrange("p (t j) -> p t j", j=T),
                        in0=frep[:, sl].rearrange("p (t j) -> p t j", j=T),
                        scalar=0.0,
                        in1=tr_v[:, b2, :].unsqueeze(1).broadcast_to([P, CH, T]),
                        op0=AL.bypass, op1=AL.add)
                ft_tiles.append(ft)

            # forward recurrence
            for t in range(1, S):
                c, tau = t // CH, t % CH
                ft = ft_tiles[c]
                cur = wpool.tile([P, 2 * T], F32, tag="cur")
                ft_t = ft[:].rearrange(
                    "p (b2 t j) -> p b2 t j", t=CH, j=T)[:, :, tau, :]
                p_prev = (parthist[:, (t - 1) * 2:(t - 1) * 2 + 2]
                          .unsqueeze(2).broadcast_to([P, 2, T]))
                nc.vector.scalar_tensor_tensor(
                    out=cur[:].rearrange("p (b2 j) -> p b2 j", j=T),
                    in0=ft_t, scalar=0.0, in1=p_prev,
                    op0=AL.bypass, op1=AL.add)
                for b2 in range(2):
                    nc.vector.tensor_reduce(
                        out=parthist[:, t * 2 + b2:t * 2 + b2 + 1],
                        in_=cur[:, b2 * T:(b2 + 1) * T],
                        axis=X, op=AL.max, apply_transpose=True)

            nc.sync.dma_start(out_d[:], parthist[:])

    nc.compile()
    return nc


def _run_device(feats, trans, **spmd_kwargs):
    """Run the SPMD forward. Returns part_hist (S, B, T) f32."""
    from concourse.bass_utils import run_bass_kernel_spmd

    if "prog" not in _PROGRAM_CACHE:
        _PROGRAM_CACHE["prog"] = _build_program()
    nc = _PROGRAM_CACHE["prog"]

    in_maps = []
    for c in range(NCORES):
        shard = np.ascontiguousarray(feats[c * BPC:(c + 1) * BPC])
        in_maps.append({"feats": shard, "trans": np.ascontiguousarray(trans)})
    res = run_bass_kernel_spmd(nc, in_maps, list(range(NCORES)), **spmd_kwargs)

    part_hist = np.empty((S, B, T), dtype=np.float32)
    for c in range(NCORES):
        ph = res.results[c]["parthist"]            # [128, S*2]
        v = ph.reshape(4, 32, S, 2)                # [b4, j, t, b2]
        part_hist[:, c * BPC:(c + 1) * BPC, :] = (
            v.transpose(2, 0, 3, 1).reshape(S, BPC, T))
    _PROGRAM_CACHE["last_results"] = res
    return part_hist


def _host_backtrack(part_hist, feats, mask, trans):
    """Reproduce the reference decode exactly from part_hist."""
    lengths = mask.astype(np.int64).sum(axis=1)
    bidx = np.arange(B)
    last_part = part_hist[lengths - 1, bidx]            # (B, T)
    last_values = last_part[:, :, None] + trans[None, :, :]
    pointer = last_values.argmax(axis=1)[:, END].astype(np.int32)

    decode = np.zeros((S, B), dtype=np.int32)
    decode[S - 1] = pointer
    ptr = pointer.copy()
    transT = np.ascontiguousarray(trans.T)              # [j, i]
    for k in range(S - 2, -1, -1):
        t = k + 1
        fcol = feats[bidx, t, ptr]                      # (B,)
        ftcol = fcol[:, None] + transT[ptr]             # fl(f+trans)
        curcol = ftcol + part_hist[t - 1, bidx]         # fl(.+part)
        bpcol = curcol.argmax(axis=1).astype(np.int32)
        newp = np.where(k == lengths - 1, pointer,
                        np.where(k > lengths - 1, 0, bpcol)).astype(np.int32)
        decode[k] = newp
        ptr = newp
    return decode.T.astype(np.int32)                    # (B, S)


def kernel(feats, mask, transitions):
    feats = np.asarray(feats, dtype=np.float32)
    mask_np = np.asarray(mask)
    trans = np.asarray(transitions, dtype=np.float32)
    part_hist = _run_device(feats, trans)
    return _host_backtrack(part_hist, feats, mask_np, trans)


# revision 11
# speedup vs baseline: 1.0921x; 1.0921x over previous
"""CRF Viterbi decode on 8 Trainium2 NeuronCores.

Strategy (data parallel over batch):
  - 64 batches sharded 8-per-core; the (T+2)^2 transition matrix replicated.
  - Each core runs the sequential Viterbi forward recurrence for its 8
    sequences entirely on the vector engine, with the exact same float
    rounding order as the reference (cur = fl(fl(feat + trans) + part)),
    producing a bitwise-identical partition history part_hist[t, b, :].
  - Per-core layout: partitions = (b4:4, tag:32); per step one fused
    scalar_tensor_tensor add builds cur[(b4,i),(b2,j)] and two
    tensor_reduce(max, apply_transpose=True) ops do the 32x32 block
    transpose + max-over-i in one instruction each, writing part_t back
    in per-partition (b4,j) layout so the recurrence needs no extra
    data movement.
  - feats are pre-combined with transitions in bulk: replicated-broadcast
    DMAs build FREP[(b4,i), (b2,tau,j)] chunks and one wide STT per chunk
    computes FT = fl(feat + trans).
  - The backpointer reconstruction (O(B*S*T) gathers/argmaxes vs the
    device's O(B*S*T^2) DP) runs on host in numpy with the identical
    rounding, reproducing the reference decode exactly, including the
    mask/length handling.
"""

import numpy as np

B, S, T = 64, 512, 32
NCORES = 8
BPC = B // NCORES          # batches per core
P = 128
START, END = T - 2, T - 1
CH = 32                    # time-chunk for FT build
NCHUNK = S // CH

_PROGRAM_CACHE = {}
import os as _os
VARIANT = _os.environ.get("CRF_VARIANT", "v1")


def _build_program():
    import concourse.mybir as mybir
    from concourse import bacc, tile

    AL = mybir.AluOpType
    F32 = mybir.dt.float32
    X = mybir.AxisListType.X

    nc = bacc.Bacc("TRN2", target_bir_lowering=False, debug=False)
    feats_d = nc.dram_tensor("feats", [BPC, S, T], F32, kind="ExternalInput").ap()
    trans_d = nc.dram_tensor("trans", [T, T], F32, kind="ExternalInput").ap()
    out_d = nc.dram_tensor("parthist", [P, S * 2], F32, kind="ExternalOutput").ap()

    with tile.TileContext(nc) as tc:
        with (
            tc.tile_pool(name="const", bufs=1) as cpool,
            tc.tile_pool(name="ft", bufs=1) as ftpool,
            tc.tile_pool(name="frep", bufs=3) as freppool,
            tc.tile_pool(name="work", bufs=2) as wpool,
        ):
            transi = cpool.tile([P, 2 * T], F32, tag="transi")
            transstart = cpool.tile([P, 1], F32, tag="transstart")
            f0t = cpool.tile([P, 2], F32, tag="f0t")
            parthist = cpool.tile([P, S * 2], F32, tag="parthist")

            # TRANSI[(b4,i),(b2,j)]=trans[i,j]; TRANSSTART[(b4,j)]=trans[START,j];
            # F0T[(b4,j),b2]=feats[b4*2+b2,0,j]  -- per-b4 2D-dest DMAs
            for b4 in range(4):
                sl = slice(b4 * 32, (b4 + 1) * 32)
                nc.sync.dma_start(
                    transi[sl, :].rearrange("p (b2 j) -> p b2 j", j=T),
                    trans_d.unsqueeze(1).broadcast_to([T, 2, T]))
                nc.sync.dma_start(
                    transstart[sl, :], trans_d[START, :].unsqueeze(1))
                nc.sync.dma_start(
                    f0t[sl, :],
                    feats_d[b4 * 2:b4 * 2 + 2, 0, :].rearrange("b j -> j b"))

            # part0 = fl(f0 + trans[START])
            nc.vector.scalar_tensor_tensor(
                out=parthist[:, 0:2], in0=f0t[:], scalar=0.0,
                in1=transstart[:].broadcast_to([P, 2]),
                op0=AL.bypass, op1=AL.add)

            # FT chunks: fl(feat + trans), i-orientation
            ft_tiles = []
            tr_v = transi[:].rearrange("p (b2 j) -> p b2 j", j=T)
            for c in range(NCHUNK):
                frep = freppool.tile([P, 2 * CH * T], F32, tag="frep")
                for b4 in range(4):
                    for b2 in range(2):
                        b = b4 * 2 + b2
                        src = (feats_d[b, c * CH:(c + 1) * CH, :]
                               .rearrange("t j -> (t j)").unsqueeze(0)
                               .broadcast_to([32, CH * T]))
                        nc.sync.dma_start(
                            frep[b4 * 32:(b4 + 1) * 32,
                                 b2 * CH * T:(b2 + 1) * CH * T], src)
                ft = ftpool.tile([P, 2 * CH * T], F32, tag=f"ft{c}")
                for b2 in range(2):
                    sl = slice(b2 * CH * T, (b2 + 1) * CH * T)
                    nc.vector.scalar_tensor_tensor(
                        out=ft[:, sl].rearrange("p (t j) -> p t j", j=T),
                        in0=frep[:, sl].rearrange("p (t j) -> p t j", j=T),
                        scalar=0.0,
                        in1=tr_v[:, b2, :].unsqueeze(1).broadcast_to([P, CH, T]),
                        op0=AL.bypass, op1=AL.add)
                ft_tiles.append(ft)

            # forward recurrence
            for t in range(1, S):
                c, tau = t // CH, t % CH
                ft = ft_tiles[c]
                cur = wpool.tile([P, 2 * T], F32, tag="cur")
                ft_t = ft[:].rearrange(
                    "p (b2 t j) -> p b2 t j", t=CH, j=T)[:, :, tau, :]
                p_prev = (parthist[:, (t - 1) * 2:(t - 1) * 2 + 2]
                          .unsqueeze(2).broadcast_to([P, 2, T]))
                nc.vector.scalar_tensor_tensor(
                    out=cur[:].rearrange("p (b2 j) -> p b2 j", j=T),
                    in0=ft_t, scalar=0.0, in1=p_prev,
                    op0=AL.bypass, op1=AL.add)
                if VARIANT == "v1":
                    # one fused 3D transpose+grouped-reduce for both b2
                    nc.vector.tensor_reduce(
                        out=parthist[:, t * 2:t * 2 + 2],
                        in_=cur[:].rearrange("p (b2 j) -> p b2 j", j=T),
                        axis=X, op=AL.max, apply_transpose=True)
                else:
                    for b2 in range(2):
                        nc.vector.tensor_reduce(
                            out=parthist[:, t * 2 + b2:t * 2 + b2 + 1],
                            in_=cur[:, b2 * T:(b2 + 1) * T],
                            axis=X, op=AL.max, apply_transpose=True)

            nc.sync.dma_start(out_d[:], parthist[:])

    nc.compile()
    return nc


def _run_device(feats, trans, **spmd_kwargs):
    """Run the SPMD forward. Returns part_hist (S, B, T) f32."""
    from concourse.bass_utils import run_bass_kernel_spmd

    if "prog" not in _PROGRAM_CACHE:
        _PROGRAM_CACHE["prog"] = _build_program()
    nc = _PROGRAM_CACHE["prog"]

    in_maps = []
    for c in range(NCORES):
        shard = np.ascontiguousarray(feats[c * BPC:(c + 1) * BPC])
        in_maps.append({"feats": shard, "trans": np.ascontiguousarray(trans)})
    res = run_bass_kernel_spmd(nc, in_maps, list(range(NCORES)), **spmd_kwargs)

    part_hist = np.empty((S, B, T), dtype=np.float32)
    for c in range(NCORES):
        ph = res.results[c]["parthist"]            # [128, S*2]
        v = ph.reshape(4, 32, S, 2)                # [b4, j, t, b2]
        part_hist[:, c * BPC:(c + 1) * BPC, :] = (
            v.transpose(2, 0, 3, 1).reshape(S, BPC, T))
    _PROGRAM_CACHE["last_results"] = res
    return part_hist


def _host_backtrack(part_hist, feats, mask, trans):
    """Reproduce the reference decode exactly from part_hist."""
    lengths = mask.astype(np.int64).sum(axis=1)
    bidx = np.arange(B)
    last_part = part_hist[lengths - 1, bidx]            # (B, T)
    last_values = last_part[:, :, None] + trans[None, :, :]
    pointer = last_values.argmax(axis=1)[:, END].astype(np.int32)

    decode = np.zeros((S, B), dtype=np.int32)
    decode[S - 1] = pointer
    ptr = pointer.copy()
    transT = np.ascontiguousarray(trans.T)              # [j, i]
    for k in range(S - 2, -1, -1):
        t = k + 1
        fcol = feats[bidx, t, ptr]                      # (B,)
        ftcol = fcol[:, None] + transT[ptr]             # fl(f+trans)
        curcol = ftcol + part_hist[t - 1, bidx]         # fl(.+part)
        bpcol = curcol.argmax(axis=1).astype(np.int32)
        newp = np.where(k == lengths - 1, pointer,
                        np.where(k > lengths - 1, 0, bpcol)).astype(np.int32)
        decode[k] = newp
        ptr = newp
    return decode.T.astype(np.int32)                    # (B, S)


def kernel(feats, mask, transitions):
    feats = np.asarray(feats, dtype=np.float32)
    mask_np = np.asarray(mask)
    trans = np.asarray(transitions, dtype=np.float32)
    part_hist = _run_device(feats, trans)
    return _host_backtrack(part_hist, feats, mask_np, trans)
